# revision 5
# baseline (speedup 1.0000x reference)
"""Trainium2 Bass kernel for nn_GCNNDiagGaussianActor.

Key structural insight: the reference GNN runs GCNConv layers over a COMPLETE
graph of 32 nodes per sample with self-loops. Every node therefore has degree
exactly 32 and the symmetric GCN normalization is the constant
norm = rsqrt(32)^2 ~= 1/32 for every edge. The gather + segment_sum message
passing collapses to a per-graph mean over nodes, broadcast back to every
node. The whole network reduces to, per graph g:

    pooled = sum_n obs[g, n, 2:16]                  (node-mean fused into W1)
    h1  = relu(pooled @ (W1 * norm) + b1)
    h2  = relu(h1 @ (W2 * 32 * norm) + b2)
    m   = relu(h2 @ Wm1 + bm1)
    o   = m @ Wm2 + bm2                              -> [4] per graph
    mu  = o[:2];  std = exp(3.5 * tanh(o[2:]) - 1.5)
    out[0, g] = tile(mu, 32); out[1, g] = tile(std, 32)

Sharding: data-parallel over the batch. 1024 graphs / 8 cores = 128 graphs
per core = exactly the 128 SBUF partitions. Weights are replicated. The x32
node replication of the output is folded into the final matmul by replicating
Wm2's columns host-side, so the last GEMM directly produces the [128, 64]
output planes in graph-major layout (no final transpose needed).
"""

import numpy as np

NCORES = 8
BS = 1024
BS_LOCAL = BS // NCORES   # 128 graphs per core
NN = 32                   # nodes per graph
FD = 16                   # per-node obs width
OBS_W = NN * FD           # 512
H = 128                   # hidden width
OUT_W = 2 * NN            # 64 = ACT_DIM * NN

_NC_CACHE = {}


def _build_bass():
    import concourse.bacc as bacc
    import concourse.mybir as mybir
    from concourse import masks, tile

    fp32 = mybir.dt.float32
    AF = mybir.ActivationFunctionType

    nc = bacc.Bacc(None, target_bir_lowering=False)
    obs = nc.declare_dram_parameter("obs", [BS_LOCAL, OBS_W], fp32, isOutput=False)
    w1p = nc.declare_dram_parameter("w1p", [FD, H], fp32, isOutput=False)
    b1 = nc.declare_dram_parameter("b1", [H, 1], fp32, isOutput=False)
    w2 = nc.declare_dram_parameter("w2", [H, H], fp32, isOutput=False)
    b2 = nc.declare_dram_parameter("b2", [H, 1], fp32, isOutput=False)
    wm1 = nc.declare_dram_parameter("wm1", [H, H], fp32, isOutput=False)
    bm1 = nc.declare_dram_parameter("bm1", [H, 1], fp32, isOutput=False)
    wm2r = nc.declare_dram_parameter("wm2r", [H, 2 * OUT_W], fp32, isOutput=False)
    bm2r = nc.declare_dram_parameter("bm2r", [1, 2 * OUT_W], fp32, isOutput=False)
    out = nc.declare_dram_parameter("out", [2, BS_LOCAL, OUT_W], fp32, isOutput=True)

    with tile.TileContext(nc) as tc:
        with (
            tc.tile_pool(name="const", bufs=1) as cpool,
            tc.tile_pool(name="work", bufs=1) as wpool,
            tc.tile_pool(name="psum", bufs=1, space="PSUM") as ppool,
        ):
            ident = cpool.tile([128, 128], fp32)
            masks.make_identity(nc, ident[:])
            ones1 = cpool.tile([1, BS_LOCAL], fp32)
            nc.vector.memset(ones1[:], 1.0)
            cm15 = cpool.tile([BS_LOCAL, 1], fp32)
            nc.vector.memset(cm15[:], -1.5)

            w1p_t = cpool.tile([FD, H], fp32)
            nc.sync.dma_start(w1p_t[:], w1p[:])
            b1_t = cpool.tile([H, 1], fp32)
            nc.sync.dma_start(b1_t[:], b1[:])
            w2_t = cpool.tile([H, H], fp32)
            nc.sync.dma_start(w2_t[:], w2[:])
            b2_t = cpool.tile([H, 1], fp32)
            nc.sync.dma_start(b2_t[:], b2[:])
            wm1_t = cpool.tile([H, H], fp32)
            nc.sync.dma_start(wm1_t[:], wm1[:])
            bm1_t = cpool.tile([H, 1], fp32)
            nc.sync.dma_start(bm1_t[:], bm1[:])
            wm2r_t = cpool.tile([H, 2 * OUT_W], fp32)
            nc.sync.dma_start(wm2r_t[:], wm2r[:])
            bm2r_t = cpool.tile([1, 2 * OUT_W], fp32)
            nc.sync.dma_start(bm2r_t[:], bm2r[:])

            obs_t = wpool.tile([BS_LOCAL, OBS_W], fp32)
            nc.sync.dma_start(obs_t[:], obs[:])

            # Sum over the 32 nodes of each graph: the 512-wide row is 32
            # node-blocks of 16; halving adds stay node-block aligned.
            s256 = wpool.tile([BS_LOCAL, 256], fp32)
            nc.vector.tensor_add(s256[:], obs_t[:, 0:256], obs_t[:, 256:512])
            s128 = wpool.tile([BS_LOCAL, 128], fp32)
            nc.vector.tensor_add(s128[:], s256[:, 0:128], s256[:, 128:256])
            s64 = wpool.tile([BS_LOCAL, 64], fp32)
            nc.vector.tensor_add(s64[:], s128[:, 0:64], s128[:, 64:128])
            s32 = wpool.tile([BS_LOCAL, 32], fp32)
            nc.vector.tensor_add(s32[:], s64[:, 0:32], s64[:, 32:64])
            s16 = wpool.tile([BS_LOCAL, FD], fp32)
            nc.vector.tensor_add(s16[:], s32[:, 0:16], s32[:, 16:32])

            # [128 graphs, 16] -> [16, 128 graphs] so the feature dim is the
            # matmul contraction (partition) dim.
            pT_ps = ppool.tile([FD, BS_LOCAL], fp32)
            nc.tensor.transpose(pT_ps[:], s16[:], ident[:])
            spT = wpool.tile([FD, BS_LOCAL], fp32)
            nc.scalar.copy(spT[:], pT_ps[:])

            # Channel-major MLP chain: [ch, graphs] tiles, weights as lhsT.
            h1_ps = ppool.tile([H, BS_LOCAL], fp32)
            nc.tensor.matmul(h1_ps[:], w1p_t[:], spT[:], start=True, stop=True)
            h1 = wpool.tile([H, BS_LOCAL], fp32)
            nc.scalar.activation(h1[:], h1_ps[:], AF.Relu, bias=b1_t[:], scale=1.0)

            h2_ps = ppool.tile([H, BS_LOCAL], fp32)
            nc.tensor.matmul(h2_ps[:], w2_t[:], h1[:], start=True, stop=True)
            h2 = wpool.tile([H, BS_LOCAL], fp32)
            nc.scalar.activation(h2[:], h2_ps[:], AF.Relu, bias=b2_t[:], scale=1.0)

            m_ps = ppool.tile([H, BS_LOCAL], fp32)
            nc.tensor.matmul(m_ps[:], wm1_t[:], h2[:], start=True, stop=True)
            m = wpool.tile([H, BS_LOCAL], fp32)
            nc.scalar.activation(m[:], m_ps[:], AF.Relu, bias=bm1_t[:], scale=1.0)

            # Final layer with node-replicated weights: lhsT = m [ch, graphs]
            # puts graphs on PSUM partitions; cols 0:64 = mu plane, 64:128 =
            # log_std plane. Bias folded in via a K=1 ones-row matmul.
            o_ps = ppool.tile([BS_LOCAL, 2 * OUT_W], fp32)
            nc.tensor.matmul(o_ps[:], m[:], wm2r_t[:], start=True, stop=False)
            nc.tensor.matmul(o_ps[:], ones1[:], bm2r_t[:], start=False, stop=True)

            mu_sb = wpool.tile([BS_LOCAL, OUT_W], fp32)
            nc.scalar.copy(mu_sb[:], o_ps[:, 0:OUT_W])
            tls = wpool.tile([BS_LOCAL, OUT_W], fp32)
            nc.scalar.activation(tls[:], o_ps[:, OUT_W : 2 * OUT_W], AF.Tanh)
            # log_std = -5 + 3.5*(tanh+1) = 3.5*tanh - 1.5; std = exp(log_std)
            std_sb = wpool.tile([BS_LOCAL, OUT_W], fp32)
            nc.scalar.activation(std_sb[:], tls[:], AF.Exp, bias=cm15[:], scale=3.5)

            nc.sync.dma_start(out[0], mu_sb[:])
            nc.sync.dma_start(out[1], std_sb[:])

    nc.compile()
    return nc


def _get_nc():
    if "nc" not in _NC_CACHE:
        _NC_CACHE["nc"] = _build_bass()
    return _NC_CACHE["nc"]


def _prep_inputs(inputs):
    obs = np.ascontiguousarray(np.asarray(inputs["obs"], dtype=np.float32))
    W1 = np.asarray(inputs["W1"], dtype=np.float32)
    b1 = np.asarray(inputs["b1"], dtype=np.float32)
    W2 = np.asarray(inputs["W2"], dtype=np.float32)
    b2 = np.asarray(inputs["b2"], dtype=np.float32)
    Wm1 = np.asarray(inputs["Wm1"], dtype=np.float32)
    bm1 = np.asarray(inputs["bm1"], dtype=np.float32)
    Wm2 = np.asarray(inputs["Wm2"], dtype=np.float32)
    bm2 = np.asarray(inputs["bm2"], dtype=np.float32)

    d = np.float32(1.0) / np.float32(np.sqrt(np.float32(32.0)))
    norm2 = np.float32(d * d)              # GCN symmetric norm, all edges
    W1p = np.zeros((FD, H), np.float32)
    W1p[2:FD] = W1 * norm2                 # drops robot_loc cols 0:2, scales
    W2s = (W2 * np.float32(np.float32(32.0) * norm2)).astype(np.float32)
    Wm2r = np.ascontiguousarray(
        np.concatenate([np.tile(Wm2[:, 0:2], NN), np.tile(Wm2[:, 2:4], NN)], axis=1)
    )
    bm2r = np.ascontiguousarray(
        np.concatenate([np.tile(bm2[0:2], NN), np.tile(bm2[2:4], NN)])[None, :]
    )
    shared = {
        "w1p": W1p,
        "b1": np.ascontiguousarray(b1.reshape(H, 1)),
        "w2": W2s,
        "b2": np.ascontiguousarray(b2.reshape(H, 1)),
        "wm1": np.ascontiguousarray(Wm1),
        "bm1": np.ascontiguousarray(bm1.reshape(H, 1)),
        "wm2r": Wm2r,
        "bm2r": bm2r,
    }
    in_maps = []
    for c in range(NCORES):
        m = dict(shared)
        m["obs"] = obs[c * BS_LOCAL : (c + 1) * BS_LOCAL]
        in_maps.append(m)
    return in_maps


def kernel(**inputs):
    from concourse.bass_utils import run_bass_kernel_spmd

    assert inputs["obs"].shape == (BS, OBS_W), inputs["obs"].shape
    nc = _get_nc()
    in_maps = _prep_inputs(inputs)
    res = run_bass_kernel_spmd(nc, in_maps, list(range(NCORES))).results
    out = np.empty((2, BS, OUT_W), np.float32)
    for c in range(NCORES):
        out[:, c * BS_LOCAL : (c + 1) * BS_LOCAL, :] = res[c]["out"]
    return out


# revision 14
# speedup vs baseline: 1.2308x; 1.2308x over previous
"""Trainium2 Bass kernel for nn_GCNNDiagGaussianActor.

Key structural insight: the reference GNN runs GCNConv layers over a COMPLETE
graph of 32 nodes per sample with self-loops. Every node therefore has degree
exactly 32 and the symmetric GCN normalization is the constant
norm = rsqrt(32)^2 ~= 1/32 for every edge. The gather + segment_sum message
passing collapses to a per-graph mean over nodes, broadcast back to every
node. The whole network reduces to, per graph g:

    pooled = sum_n obs[g, n, 2:16]                  (node-mean fused into W1)
    h1  = relu(pooled @ (W1 * norm) + b1)
    h2  = relu(h1 @ (W2 * 32 * norm) + b2)
    m   = relu(h2 @ Wm1 + bm1)
    o   = m @ Wm2 + bm2                              -> [4] per graph
    mu  = o[:2];  std = exp(3.5 * tanh(o[2:]) - 1.5)
    out[0, g] = tile(mu, 32); out[1, g] = tile(std, 32)

Sharding: data-parallel over the batch. 1024 graphs / 8 cores = 128 graphs
per core = exactly the 128 SBUF partitions. Weights are replicated. The x32
node replication of the output is folded into the final matmul by replicating
Wm2's columns host-side, so the last GEMM directly produces the [128, 64]
output planes in graph-major layout.

Perf notes (v2): inputs ride in 3 DMAs (obs; one packed weight tensor; one
tiny [17,128] with W1p + bm2 row) to cut per-DMA fixed costs; node pooling is
a single strided tensor_reduce; the pooled [128,16] -> [16,128] transpose
uses 4 DVE 32x32 block transposes (no identity matrix, no gpsimd, no PSUM
round-trip); relu+bias is fused on the vector engine via tensor_scalar; the
mu-plane output DMA issues while std's tanh/exp still run.
"""

import numpy as np

NCORES = 8
BS = 1024
BS_LOCAL = BS // NCORES   # 128 graphs per core
NN = 32                   # nodes per graph
FD = 16                   # per-node obs width
OBS_W = NN * FD           # 512
H = 128                   # hidden width
OUT_W = 2 * NN            # 64 = ACT_DIM * NN

_NC_CACHE = {}


def _build_bass():
    import concourse.bacc as bacc
    import concourse.mybir as mybir
    from concourse import tile

    fp32 = mybir.dt.float32
    AF = mybir.ActivationFunctionType
    ALU = mybir.AluOpType

    nc = bacc.Bacc(None, target_bir_lowering=False)
    obs = nc.declare_dram_parameter("obs", [BS_LOCAL, OBS_W], fp32, isOutput=False)
    # packed weights: cols 0:128 W2s | 128:256 Wm1 | 256:384 Wm2r | 384 b1 |
    # 385 b2 | 386 bm1
    wpack = nc.declare_dram_parameter("wpack", [H, 3 * H + 3], fp32, isOutput=False)
    # W1p in lhsT layout; node-replicated bm2 separate (matmul operands must
    # share base partition 0, so bm2r gets its own partition-0 tile)
    w1b = nc.declare_dram_parameter("w1b", [FD, H], fp32, isOutput=False)
    bm2r = nc.declare_dram_parameter("bm2r", [1, H], fp32, isOutput=False)
    out = nc.declare_dram_parameter("out", [2, BS_LOCAL, OUT_W], fp32, isOutput=True)

    with tile.TileContext(nc) as tc:
        with (
            tc.tile_pool(name="sb", bufs=1) as pool,
            tc.tile_pool(name="ps", bufs=1, space="PSUM") as ppool,
        ):
            obs_t = pool.tile([BS_LOCAL, OBS_W], fp32)
            nc.sync.dma_start(obs_t[:], obs[:])
            wp = pool.tile([H, 3 * H + 3], fp32)
            nc.sync.dma_start(wp[:], wpack[:])
            w1b_t = pool.tile([FD, H], fp32)
            nc.sync.dma_start(w1b_t[:], w1b[:])
            bm2r_t = pool.tile([1, H], fp32)
            nc.sync.dma_start(bm2r_t[:], bm2r[:])

            ones1 = pool.tile([1, BS_LOCAL], fp32)
            nc.vector.memset(ones1[:], 1.0)
            cm15 = pool.tile([BS_LOCAL, 1], fp32)
            nc.vector.memset(cm15[:], -1.5)

            # Node pooling: obs row is 32 node-blocks of 16 features; one
            # strided reduce sums over nodes -> S[:, 0:16].
            S = pool.tile([BS_LOCAL, 2 * FD], fp32)
            nc.vector.memset(S[:], 0.0)
            nc.vector.tensor_reduce(
                S[:, 0:FD],
                obs_t[:].rearrange("p (n c) -> p c n", c=FD),
                axis=mybir.AxisListType.X,
                op=ALU.add,
            )
            # [128, 16] -> [16, 128] via DVE 32x32 block transposes (rows
            # 16:32 of T are the transposed zero padding, never read).
            T = pool.tile([2 * FD, BS_LOCAL], fp32)
            for b in range(4):
                nc.vector.transpose(
                    T[:, 32 * b : 32 * (b + 1)], S[32 * b : 32 * (b + 1), :]
                )

            # Channel-major MLP chain: [ch, graphs] tiles, weights as lhsT,
            # relu+bias fused on DVE (out = max(psum + b, 0)).
            h1_ps = ppool.tile([H, BS_LOCAL], fp32)
            nc.tensor.matmul(h1_ps[:], w1b_t[0:FD, :], T[0:FD, :], start=True, stop=True)
            h1 = pool.tile([H, BS_LOCAL], fp32)
            nc.vector.tensor_scalar(
                h1[:], h1_ps[:], wp[:, 384:385], 0.0, ALU.add, ALU.max
            )

            h2_ps = ppool.tile([H, BS_LOCAL], fp32)
            nc.tensor.matmul(h2_ps[:], wp[:, 0:H], h1[:], start=True, stop=True)
            h2 = pool.tile([H, BS_LOCAL], fp32)
            nc.vector.tensor_scalar(
                h2[:], h2_ps[:], wp[:, 385:386], 0.0, ALU.add, ALU.max
            )

            m_ps = ppool.tile([H, BS_LOCAL], fp32)
            nc.tensor.matmul(m_ps[:], wp[:, H : 2 * H], h2[:], start=True, stop=True)
            m = pool.tile([H, BS_LOCAL], fp32)
            nc.vector.tensor_scalar(
                m[:], m_ps[:], wp[:, 386:387], 0.0, ALU.add, ALU.max
            )

            # Final layer with node-replicated weights: lhsT = m [ch, graphs]
            # puts graphs on PSUM partitions; cols 0:64 = mu plane, 64:128 =
            # log_std plane. bm2 folded in via a K=1 ones-row matmul.
            o_ps = ppool.tile([BS_LOCAL, 2 * OUT_W], fp32)
            nc.tensor.matmul(o_ps[:], m[:], wp[:, 2 * H : 3 * H], start=True, stop=False)
            nc.tensor.matmul(
                o_ps[:], ones1[:], bm2r_t[:], start=False, stop=True
            )

            O = pool.tile([BS_LOCAL, 2 * OUT_W], fp32)
            nc.vector.tensor_copy(O[:, 0:OUT_W], o_ps[:, 0:OUT_W])
            nc.sync.dma_start(out[0], O[:, 0:OUT_W])

            # log_std = -5 + 3.5*(tanh+1) = 3.5*tanh - 1.5; std = exp(log_std)
            tls = pool.tile([BS_LOCAL, OUT_W], fp32)
            nc.scalar.activation(tls[:], o_ps[:, OUT_W : 2 * OUT_W], AF.Tanh)
            nc.scalar.activation(
                O[:, OUT_W : 2 * OUT_W], tls[:], AF.Exp, bias=cm15[:], scale=3.5
            )
            nc.sync.dma_start(out[1], O[:, OUT_W : 2 * OUT_W])

    nc.compile()
    return nc


def _get_nc():
    if "nc" not in _NC_CACHE:
        _NC_CACHE["nc"] = _build_bass()
    return _NC_CACHE["nc"]


def _prep_inputs(inputs):
    obs = np.ascontiguousarray(np.asarray(inputs["obs"], dtype=np.float32))
    W1 = np.asarray(inputs["W1"], dtype=np.float32)
    b1 = np.asarray(inputs["b1"], dtype=np.float32)
    W2 = np.asarray(inputs["W2"], dtype=np.float32)
    b2 = np.asarray(inputs["b2"], dtype=np.float32)
    Wm1 = np.asarray(inputs["Wm1"], dtype=np.float32)
    bm1 = np.asarray(inputs["bm1"], dtype=np.float32)
    Wm2 = np.asarray(inputs["Wm2"], dtype=np.float32)
    bm2 = np.asarray(inputs["bm2"], dtype=np.float32)

    d = np.float32(1.0) / np.float32(np.sqrt(np.float32(32.0)))
    norm2 = np.float32(d * d)              # GCN symmetric norm, all edges
    W1p = np.zeros((FD, H), np.float32)
    W1p[2:FD] = W1 * norm2                 # drops robot_loc cols 0:2, scales
    W2s = (W2 * np.float32(np.float32(32.0) * norm2)).astype(np.float32)
    Wm2r = np.concatenate([np.tile(Wm2[:, 0:2], NN), np.tile(Wm2[:, 2:4], NN)], axis=1)
    bm2r = np.concatenate([np.tile(bm2[0:2], NN), np.tile(bm2[2:4], NN)])

    wpack = np.ascontiguousarray(
        np.concatenate(
            [W2s, Wm1, Wm2r, b1[:, None], b2[:, None], bm1[:, None]], axis=1
        ).astype(np.float32)
    )
    shared = {
        "wpack": wpack,
        "w1b": np.ascontiguousarray(W1p),
        "bm2r": np.ascontiguousarray(bm2r[None, :]),
    }
    in_maps = []
    for c in range(NCORES):
        mm = dict(shared)
        mm["obs"] = obs[c * BS_LOCAL : (c + 1) * BS_LOCAL]
        in_maps.append(mm)
    return in_maps


def kernel(**inputs):
    from concourse.bass_utils import run_bass_kernel_spmd

    assert inputs["obs"].shape == (BS, OBS_W), inputs["obs"].shape
    nc = _get_nc()
    in_maps = _prep_inputs(inputs)
    res = run_bass_kernel_spmd(nc, in_maps, list(range(NCORES))).results
    out = np.empty((2, BS, OUT_W), np.float32)
    for c in range(NCORES):
        out[:, c * BS_LOCAL : (c + 1) * BS_LOCAL, :] = res[c]["out"]
    return out
